# revision 22
# baseline (speedup 1.0000x reference)
"""Distributed Trainium2 kernel for a pre-LN single attention block.

Reference computation (dims hardcoded):
    x: [4, 2048, 1024]; LN(x) -> q = xn@Wq, kv = xn@Wkv; 16 heads x 64;
    softmax(q k^T / 8) v ; out proj [1024,1024] + bias.

Sharding over 8 NeuronCores: core c handles batch b = c//2 and head
group g = c%2 (8 heads each).  Each core computes LN(x[b]), its
512-wide q/k/v projection slices, its 8 attention heads and a PARTIAL
out-projection; the two partials per batch are summed on the host.
gamma is folded into the projection weights on the host.

The kernel is organised around the ACT engine: the 256 exp
instructions ([128,1024] each, ~1.15us) are the hard bottleneck, so
every other engine's work is scheduled to hide under that stream.

  - Attention runs pair-of-heads at a time (heads 2m/2m+1 occupy SBUF
    partitions 0-63/64-127 of the qT/kT tiles), with the two heads'
    scores matmuls issued as concurrent PE row-tiles T0/T8.
  - Loop nest: PAIRS outer, i-axis QUARTERS (512) inner.  Pair 0 only
    needs its own k/q projection before starting, so the exp stream
    starts ~15us in; everything else (LayerNorm of tiles 4-15,
    transposes, v projection, k/q chunks for later pairs, out-
    projection tiles of完成 quarters) is fed through per-unit
    background FIFOs drained a few items per j-step, sized to the
    PE slack under the exp stream.
  - attn@v keeps the softmax-denominator ones-column (M=65) and is
    accumulated per head in a single PSUM bank.  Head B's attn@v for
    j where v arrives late is emitted lazily (deadlock-safe order).
  - PSUM budget (8 banks): scores double-buffer 2x[128,1024] = 4,
    attn@v accumulators 2x[65,512] = 2, shared background pool
    [128,512]x2 = 2 (becomes the out-projection pool after all
    projections are done).
  - softmax normalization: z row -> DRAM round-trip broadcast ->
    reciprocal_approx_fast (the exact DVE reciprocal costs 6.4
    cyc/elem; approx is ~5x faster at 18 bits) -> one multiply.
  - xn transposes are regular identity matmuls (f32 PSUM), ~2x
    faster than transpose-mode and they warm the PE clock gate.
"""

import numpy as np
from contextlib import ExitStack

import concourse.bass as bass
import concourse.bacc as bacc_mod
import concourse.mybir as mybir
import concourse.tile as tile
from concourse.bass_utils import run_bass_kernel_spmd
from concourse.masks import make_identity

F32 = mybir.dt.float32
BF16 = mybir.dt.bfloat16
AF = mybir.ActivationFunctionType

B = 4
N = 2048          # sequence length
D = 1024          # model dim
GC = 512          # per-core inner columns (8 heads x 64)
DH = 64           # head dim
HPC = 8           # heads per core
P = 128
NT_I = N // P     # 16 sequence tiles
NT_C = D // P     # 8 model-dim tiles
NT_G = GC // P    # 4 inner tiles (= head pairs)
NQ = 4            # i-axis quarters
QW = N // NQ      # 512: quarter width
SCALE = DH ** -0.5
EPS = 1e-5
VW = HPC * (DH + 1)  # 520: v tile width incl. ones columns

LAST_EXEC_NS = None
LAST_TRACE = None
_CACHED_NC = None


def build_nc():
    nc = bacc_mod.Bacc()
    x_d = nc.declare_dram_parameter("x", [N, D], BF16, isOutput=False)
    wq_d = nc.declare_dram_parameter("wq", [D, GC], BF16, isOutput=False)
    wk_d = nc.declare_dram_parameter("wk", [D, GC], BF16, isOutput=False)
    wv_d = nc.declare_dram_parameter("wv", [D, GC], BF16, isOutput=False)
    wo_d = nc.declare_dram_parameter("wout", [GC, D], BF16, isOutput=False)
    bo_d = nc.declare_dram_parameter("bout", [1, D], F32, isOutput=False)
    out_d = nc.declare_dram_parameter("out", [N, D], F32, isOutput=True)
    zs_d = nc.dram_tensor("zscratch", [2 * HPC * NQ, QW], F32)

    ctx = ExitStack()
    with ctx:
        tc = ctx.enter_context(tile.TileContext(nc))

        # ---- pools live for the whole kernel -----------------------------
        const = ctx.enter_context(tc.tile_pool(name="const", bufs=1))
        wpool = ctx.enter_context(tc.tile_pool(name="wpool", bufs=1))
        small = ctx.enter_context(tc.tile_pool(name="small", bufs=4))
        ao_pool = ctx.enter_context(tc.tile_pool(name="aoT", bufs=1))
        qk_pool = ctx.enter_context(tc.tile_pool(name="qk", bufs=1))
        v_pool = ctx.enter_context(tc.tile_pool(name="vext", bufs=1))
        nrm_pool = ctx.enter_context(tc.tile_pool(name="nrm", bufs=1))
        y_pool = ctx.enter_context(tc.tile_pool(name="ybuf", bufs=3))
        xstage_cm = ctx.enter_context(tc.tile_pool(name="xstage", bufs=6))

        identity = const.tile([P, P], BF16, tag="identity")
        make_identity(nc, identity)
        eps_sb = const.tile([P, 1], F32, tag="eps")
        nc.vector.memset(eps_sb, EPS)
        bout_sb = const.tile([P, D], F32, tag="bout")
        nc.gpsimd.dma_start(out=bout_sb, in_=bo_d[0:1, :].to_broadcast((P, D)))

        aoT_bf = [ao_pool.tile([P, N], BF16, tag=f"ao{t}", name=f"ao{t}")
                  for t in range(NT_G)]

        # ---- weights arrive pre-cast to bf16 from the host ---------------
        def load_w(dram, rows, cols, tagp):
            tiles = []
            for t in range(rows // P):
                bf = wpool.tile([P, cols], BF16, tag=f"{tagp}{t}")
                nc.gpsimd.dma_start(out=bf, in_=dram[t * P:(t + 1) * P, :])
                tiles.append(bf)
            return tiles

        wq_bf = load_w(wq_d, D, GC, "wq")
        wk_bf = load_w(wk_d, D, GC, "wk")
        wv_bf = load_w(wv_d, D, GC, "wv")
        wo_bf = load_w(wo_d, GC, D, "wo")

        # xnT and the shared background PSUM pool live on the RIGHT
        # allocation stacks (their lifetimes aren't nested with the
        # attention pools on the left stacks).
        xnT_cm = tc.tile_pool(name="xnT", bufs=1, side="right")
        xnT_pool = xnT_cm.__enter__()
        bgps_cm = tc.tile_pool(name="bgps", bufs=2, space="PSUM", side="right")
        bgps = bgps_cm.__enter__()

        # xnT_all packs the 8 c-tiles side by side: segment ct covers
        # columns [ct*N, (ct+1)*N).
        xnT_all = xnT_pool.tile([P, NT_C * N], BF16, tag="xnT", name="xnT")

        xn_bf = [None] * NT_I
        v_ext = [None] * NT_I

        # ---- work-item emitters ------------------------------------------
        def emit_ln(i):
            xs = xstage_cm.tile([P, D], BF16, tag="xst")
            nc.sync.dma_start(out=xs, in_=x_d[i * P:(i + 1) * P, :])
            stats = small.tile([P, 2, 6], F32, tag="stats")
            for sg in range(2):
                nc.vector.bn_stats(out=stats[:, sg, :],
                                   in_=xs[:, sg * 512:(sg + 1) * 512])
            mv = small.tile([P, 2], F32, tag="mv")
            nc.vector.bn_aggr(out=mv, in_=stats)
            std = small.tile([P, 1], F32, tag="std")
            nc.scalar.activation(out=std, in_=mv[:, 1:2], func=AF.Sqrt, bias=eps_sb)
            rstd = small.tile([P, 1], F32, tag="rstd")
            nc.vector.reciprocal(out=rstd, in_=std)
            nbias = small.tile([P, 1], F32, tag="nbias")
            nc.vector.scalar_tensor_tensor(nbias, mv[:, 0:1], -1.0, rstd,
                                           op0=mybir.AluOpType.mult,
                                           op1=mybir.AluOpType.mult)
            nc.scalar.activation(out=xs, in_=xs, func=AF.Identity,
                                 bias=nbias, scale=rstd)
            xn_bf[i] = xs

        def emit_tr(i):
            # transpose xn[i] -> xnT columns, via identity matmuls,
            # in two 4-ct chunks through the shared background pool
            for half in range(2):
                ps = bgps.tile([P, 512], F32, tag="bg")
                for c4 in range(4):
                    ct = half * 4 + c4
                    nc.tensor.matmul(ps[:, c4 * P:(c4 + 1) * P],
                                     xn_bf[i][:, ct * P:(ct + 1) * P],
                                     identity, start=True, stop=True)
                nc.vector.tensor_copy(
                    out=xnT_all[:, :].rearrange("p (ct i) -> p ct i", ct=NT_C)[:, half * 4:half * 4 + 4, i * P:(i + 1) * P],
                    in_=ps[:].rearrange("p (c4 i) -> p c4 i", i=P))

        def emit_v(i):
            vt = v_pool.tile([P, VW], BF16, tag=f"v{i}", name=f"v{i}")
            nc.gpsimd.memset(vt, 1.0)
            psv = bgps.tile([P, 512], F32, tag="bg")
            for ct in range(NT_C):
                nc.tensor.matmul(psv,
                                 xnT_all[:, ct * N + i * P:ct * N + (i + 1) * P],
                                 wv_bf[ct],
                                 start=(ct == 0), stop=(ct == NT_C - 1))
            nc.vector.tensor_copy(
                out=vt[:, 0:VW].rearrange("p (h e) -> p h e", h=HPC)[:, :, 0:DH],
                in_=psv[:].rearrange("p (h e) -> p h e", e=DH))
            v_ext[i] = vt

        qT_bf = [qk_pool.tile([P, N], BF16, tag=f"qT{m}", name=f"qT{m}")
                 for m in range(NT_G)]
        kT_bf = [qk_pool.tile([P, N], BF16, tag=f"kT{m}", name=f"kT{m}")
                 for m in range(NT_G)]

        def proj_chunk(w_bf, ot, m, nck):
            ps = bgps.tile([P, 512], F32, tag="bg")
            for ct in range(NT_C):
                nc.tensor.matmul(ps,
                                 w_bf[ct][:, m * P:(m + 1) * P],
                                 xnT_all[:, ct * N + nck * 512:ct * N + (nck + 1) * 512],
                                 start=(ct == 0), stop=(ct == NT_C - 1))
            nc.vector.tensor_copy(out=ot[:, nck * 512:(nck + 1) * 512], in_=ps)

        def emit_k(m, nck):
            proj_chunk(wk_bf, kT_bf[m], m, nck)

        def emit_q(m, nck):
            proj_chunk(wq_bf, qT_bf[m], m, nck)

        psY = None

        def emit_outproj_tile(q, it):
            i0 = q * QW + it * P
            ys = y_pool.tile([P, D], F32, tag="ys")
            for nck in range(2):
                psy = psY.tile([P, 512], F32, tag=f"y{nck}")
                for t in range(NT_G):
                    nc.tensor.matmul(psy,
                                     aoT_bf[t][:, i0:i0 + P],
                                     wo_bf[t][:, nck * 512:(nck + 1) * 512],
                                     start=(t == 0), stop=(t == NT_G - 1))
                nc.vector.tensor_add(ys[:, nck * 512:(nck + 1) * 512], psy,
                                     bout_sb[:, nck * 512:(nck + 1) * 512])
            nc.sync.dma_start(out=out_d[i0:i0 + P, :], in_=ys)

        # ---- prologue: just enough to start the pair-0 exp stream --------
        for i in range(4):
            emit_ln(i)
        for i in range(4):
            emit_tr(i)
            emit_v(i)
        emit_k(0, 0)
        emit_q(0, 0)

        # ---- per-unit background schedules -------------------------------
        def LN(i):
            return lambda: emit_ln(i)

        def TR(i):
            return lambda: emit_tr(i)

        def V(i):
            return lambda: emit_v(i)

        def K(m, c):
            return lambda: emit_k(m, c)

        def Q(m, c):
            return lambda: emit_q(m, c)

        def OP(q, it):
            return lambda: emit_outproj_tile(q, it)

        s00 = []
        for i in range(4, 16):
            s00 += [LN(i), TR(i)]
            if i in (7, 11, 15):
                s00.append(K(0, (i + 1) // 4 - 1))
        s00 += [V(i) for i in range(4, 10)]
        s00.append(Q(0, 1))
        s00 += [V(i) for i in range(10, 16)]

        sched = {
            (0, 0): s00,
            (0, 1): [Q(0, 2), K(1, 0), K(1, 1)],
            (0, 2): [Q(0, 3), K(1, 2), K(1, 3)],
            (0, 3): [Q(1, 0), K(2, 0)],
            (1, 0): [Q(1, 1), K(2, 1), K(2, 2)],
            (1, 1): [Q(1, 2), K(2, 3), K(3, 0)],
            (1, 2): [Q(1, 3), K(3, 1)],
            (1, 3): [Q(2, 0), K(3, 2), K(3, 3)],
            (2, 0): [Q(2, 1), Q(3, 0)],
            (2, 1): [Q(2, 2), Q(3, 1)],
            (2, 2): [Q(2, 3), Q(3, 2)],
            (2, 3): [Q(3, 3)],
            (3, 0): [],
            (3, 1): [OP(0, 0), OP(0, 1), OP(0, 2)],
            (3, 2): [OP(0, 3), OP(1, 0), OP(1, 1), OP(1, 2)],
            (3, 3): [OP(1, 3), OP(2, 0), OP(2, 1), OP(2, 2)],
        }

        # ---- attention PSUM pools (left stack) ---------------------------
        psS_cm = tc.tile_pool(name="psS", bufs=1, space="PSUM")
        psS = psS_cm.__enter__()
        psO_cm = tc.tile_pool(name="psO", bufs=1, space="PSUM")
        psO = psO_cm.__enter__()
        pt_cm = tc.tile_pool(name="pt", bufs=1)
        pt_pool = pt_cm.__enter__()

        def normalize(o_ps, pair, head_in_pair, q):
            slot = (q * NT_G + pair) * 2 + head_in_pair
            stage = nrm_pool.tile([DH + 1, QW], F32, tag=f"st{head_in_pair}")
            nc.vector.tensor_copy(out=stage, in_=o_ps)
            nc.sync.dma_start(out=zs_d[slot:slot + 1, :], in_=stage[DH:DH + 1, :])
            zb = nrm_pool.tile([DH, QW], F32, tag=f"zb{head_in_pair}")
            nc.sync.dma_start(out=zb,
                              in_=zs_d[slot:slot + 1, :].to_broadcast((DH, QW)))
            rb = nrm_pool.tile([DH, QW], F32, tag=f"rb{head_in_pair}")
            nc.vector.reciprocal_approx_fast(out=rb, in_=zb)
            po = head_in_pair * DH
            nc.vector.tensor_mul(
                aoT_bf[pair][po:po + DH, q * QW:(q + 1) * QW],
                stage[0:DH, :], rb)

        for pair in range(NT_G):
            kt, qt = kT_bf[pair], qT_bf[pair]
            hA, hB = 2 * pair, 2 * pair + 1
            for q in range(NQ):
                unit_bg = list(sched[(pair, q)])
                L = len(unit_bg)
                done = 0
                oA = psO.tile([DH + 1, QW], F32, tag="oa", name=f"oA{pair}_{q}")
                oB = psO.tile([DH + 1, QW], F32, tag="ob", name=f"oB{pair}_{q}")
                pts = [None] * NT_I
                next_av = 0  # next j whose attn@v is pending (in-order)

                def drain_attnv(up_to_j):
                    nonlocal next_av
                    while (next_av <= up_to_j and pts[next_av] is not None
                           and v_ext[next_av] is not None):
                        jj = next_av
                        nc.tensor.matmul(
                            oA, v_ext[jj][:, hA * (DH + 1):(hA + 1) * (DH + 1)],
                            pts[jj][:, 0:512],
                            start=(jj == 0), stop=(jj == NT_I - 1),
                            skip_group_check=True)
                        nc.tensor.matmul(
                            oB, v_ext[jj][:, hB * (DH + 1):(hB + 1) * (DH + 1)],
                            pts[jj][:, 512:1024],
                            start=(jj == 0), stop=(jj == NT_I - 1),
                            skip_group_check=True)
                        next_av += 1

                for j in range(NT_I):
                    ps = psS.tile([P, 2 * 512], F32, tag=f"s{j % 2}")
                    nc.tensor.matmul(ps[:, 0:512],
                                     kt[0:DH, j * P:(j + 1) * P],
                                     qt[0:DH, q * QW:(q + 1) * QW],
                                     start=True, stop=True)
                    nc.tensor.matmul(ps[:, 512:1024],
                                     kt[DH:P, j * P:(j + 1) * P],
                                     qt[DH:P, q * QW:(q + 1) * QW],
                                     start=True, stop=True)
                    pt = pt_pool.tile([P, 2 * 512], BF16, tag=f"pt{j % 8}")
                    nc.scalar.activation(out=pt, in_=ps, func=AF.Exp, scale=SCALE)
                    pts[j] = pt
                    # paced background drain, then any attn@v now unblocked
                    target = (L * (j + 1) + NT_I - 1) // NT_I
                    while done < target and unit_bg:
                        unit_bg.pop(0)()
                        done += 1
                    drain_attnv(j)
                drain_attnv(NT_I - 1)
                assert next_av == NT_I
                normalize(oA, pair, 0, q)
                normalize(oB, pair, 1, q)

                if (pair, q) == (2, 3):
                    # all projections done: swap bgps/xnT for the
                    # out-projection pool on the right stacks
                    bgps_cm.__exit__(None, None, None)
                    xnT_cm.__exit__(None, None, None)
                    psY_cm = tc.tile_pool(name="psY", bufs=1, space="PSUM",
                                          side="right")
                    psY = psY_cm.__enter__()

        # final out-projection tiles
        emit_outproj_tile(2, 3)
        for it in range(NQ):
            emit_outproj_tile(3, it)

        psY_cm.__exit__(None, None, None)
        pt_cm.__exit__(None, None, None)
        psO_cm.__exit__(None, None, None)
        psS_cm.__exit__(None, None, None)

    nc.compile()
    return nc


def kernel(x, gamma, Wq, Wkv, Wout, bout, _trace=False, _tmpdir=None):
    global _CACHED_NC, LAST_EXEC_NS, LAST_TRACE
    x = np.asarray(x, dtype=np.float32)
    gamma = np.asarray(gamma, dtype=np.float32)
    Wq = np.asarray(Wq, dtype=np.float32)
    Wkv = np.asarray(Wkv, dtype=np.float32)
    Wout = np.asarray(Wout, dtype=np.float32)
    bout = np.asarray(bout, dtype=np.float32)

    # fold LN gamma into the projection weights (exact), cast to bf16
    import ml_dtypes
    bf = ml_dtypes.bfloat16
    Wqg = (gamma[:, None] * Wq).astype(bf)
    Wk = (gamma[:, None] * Wkv[:, :D]).astype(bf)
    Wv = (gamma[:, None] * Wkv[:, D:]).astype(bf)
    Wo_b = Wout.astype(bf)
    x_b = x.astype(bf)
    zeros_b = np.zeros((1, D), dtype=np.float32)

    in_maps = []
    for c in range(8):
        b, g = divmod(c, 2)
        sl = slice(g * GC, (g + 1) * GC)
        in_maps.append({
            "x": np.ascontiguousarray(x_b[b]),
            "wq": np.ascontiguousarray(Wqg[:, sl]),
            "wk": np.ascontiguousarray(Wk[:, sl]),
            "wv": np.ascontiguousarray(Wv[:, sl]),
            "wout": np.ascontiguousarray(Wo_b[sl, :]),
            "bout": bout.reshape(1, D) if g == 0 else zeros_b,
        })

    if _CACHED_NC is None:
        _CACHED_NC = build_nc()
    nc = _CACHED_NC

    kw = {}
    if _trace:
        import concourse.bass_utils as bu
        bu.upload_artifacts = lambda tmpdir: "not-uploaded"
        kw = dict(trace=True, tmpdir=_tmpdir)
    try:
        res = run_bass_kernel_spmd(nc, in_maps, core_ids=list(range(8)), **kw)
    except Exception:
        # transient device faults (e.g. NRT_EXEC_UNIT_UNRECOVERABLE) clear on
        # a fresh attempt; retry once before giving up
        res = run_bass_kernel_spmd(nc, in_maps, core_ids=list(range(8)), **kw)
    LAST_EXEC_NS = res.exec_time_ns
    LAST_TRACE = getattr(res, "instructions_and_trace", None)

    out = np.empty((B, N, D), dtype=np.float32)
    for b in range(B):
        out[b] = res.results[2 * b]["out"] + res.results[2 * b + 1]["out"]
    return out
